# revision 2
# baseline (speedup 1.0000x reference)
"""Fused multi-head attention + LayerNorm kernel for 8 Trainium2 NeuronCores.

Problem (hardcoded): B=4, S=2048, DIM=1024, H=16, HD=64; out = LayerNorm(
softmax(q W_q^T (k W_k^T)^T / sqrt(HD)) (v W_v^T) W_o^T + b_o) per reference.

Sharding: core c -> batch b = c//2, head-group g = c%2 (8 heads / 512 features).
Each core attends its 8 heads over all 2048 tokens; the pair exchanges
normalized attention outputs (AllReduce over pairs, sum-minus-mine) so each
core finalizes half of the tokens through the output projection + LayerNorm.

Design notes:
  - All-bf16 datapath (PSUM accumulates f32); khT/qhT stay resident in SBUF.
  - Attention inner loop is software-pipelined: scores for jt+2 issue before
    AV of jt, so the PE never waits on the ScalarE exp; psc double-buffered,
    et triple-buffered; PE runs back-to-back (HAM stays warm).
  - Softmax denominator via a ones-column in the packed V operand (row 64 of
    the AV accumulator); evict = copy row to SBUF -> reciprocal_approx_fast
    (the custom-DVE op cannot read PSUM) -> partition_broadcast -> fused
    multiply on eviction into out_normT.
  - Peer token-half (cols TH:S) is attended first for all heads, the pair
    AllReduce fires, then the own half runs; the peer reconstruction is
    emitted only after own-half head 4 so the DVE never head-of-line blocks
    on the collective (a stall there re-throttles the PE clock down to
    1.2 GHz for the rest of the kernel).
  - Output projection reads [mine (SBUF), peer] with Wo rows rotated
    per-core host-side; LayerNorm via bn_stats/bn_aggr on the DVE.

Host tricks: each core's q tokens are permuted so "my half" is always columns
0:1024; Wo rows are rotated per core so the [mine, peer] chunk order matches.

v15: qh is pre-scaled by SCALE/4 at projection eviction (bq pre-scaled on
host) so psc = score*SCALE/4, and the softmax exp is split across two
engines: ACT exp(4x) for 13/16 key-chunks and a runtime-registered custom
8-op DVE instruction ((1+u+c2u^2+c3u^3)^2)^2 ~= exp(4u) (max rel err ~1%)
for jt in {6,10,14}. This relieves the ACT engine (v1: 271us of exp against
a ~285us attention window) without perturbing the v1 pipeline, which proved
extremely sensitive: more aggressive restructurings (row-tiled score pairs,
deferred evicts, reordered emission, 6-tile DVE splits, final-phase
rewrites) all re-throttled the PE clock to 1.2 GHz via HAM and regressed.
462.5us -> 458.2us measured.
"""
import sys

sys.path.insert(0, "/opt/trn_rl_repo")

import numpy as np
import ml_dtypes

B, S, DIM, H, HD = 4, 2048, 1024, 16, 64
NCORES = 8
NH = 8             # heads per core
FL = NH * HD       # 512 local features
EPS = 1e-5
SCALE = HD ** -0.5
P = 128
JT = S // P        # 16
IC = S // 512      # 4
TH = S // 2        # 1024 tokens finalized per core
DC = DIM // P      # 8 contraction chunks
HW = HD + 1        # 65: head block width in vh_aug
VW = NH * HW       # 520 (stationary windows spill into next block)

EXP_C2 = 0.5126914
EXP_C3 = 0.16625791

_cache = {}


def _register_exp_op():
    """Custom DVE op  out = ((1+u+c2 u^2+c3 u^3)^2)^2  ~= exp(4u), |u|<=0.66."""
    from concourse import dve_ops as DO
    from concourse.dve_spec import (
        Spec, Src0, One, C0, C1, lower, _has_src1 as has_src1)
    from concourse.dve_uop import DveOpSpec
    if hasattr(DO, "EXP4SQ_ANT"):
        return DO.EXP4SQ_ANT
    name = "EXP4SQ_ANT"
    u2 = Src0 * Src0
    t = C0 + Src0 * C1
    q0 = One + Src0
    r = q0 + u2 * t
    ssq = r * r
    body = ssq * ssq

    def ref(in0, in1, s0, s1, imm2):
        p = 1 + in0 + s0 * in0 * in0 + s1 * in0 ** 3
        p = p * p
        return p * p

    spec = Spec(body=body, reference=ref)
    row = DO._CUSTOM_DVE_ROW_BASE + len(DO.OPS)
    assert row < 0x20
    DO._SUB_OPCODE_FOR_NAME[name] = row
    shas = {}
    for ver in ("v3", "v4"):
        sp = DveOpSpec(name=name, opcode=row, uops=lower(spec, ver=ver),
                       rd1_en=has_src1(spec))
        shas[ver] = sp.sha(ver)
    op = DO.DveOp(name, spec, subdim=False, uops_sha=shas)
    DO.OPS.append(op)
    DO.EXP4SQ_ANT = op
    return op


def _build(trivial_ln=False):
    import concourse.bass as bass
    import concourse.bacc as bacc
    import concourse.tile as tile
    from concourse import mybir
    f32 = mybir.dt.float32
    f32r = mybir.dt.float32r
    bf16 = mybir.dt.bfloat16
    EXPF = mybir.ActivationFunctionType.Exp
    IDENT = mybir.ActivationFunctionType.Identity
    SQRTF = mybir.ActivationFunctionType.Sqrt
    SQUARE = mybir.ActivationFunctionType.Square
    ALU = mybir.AluOpType
    exp_op = _register_exp_op()

    nc = bacc.Bacc("TRN2", target_bir_lowering=False, debug=False,
                   num_devices=NCORES)

    xqT_d = nc.dram_tensor("xqT", [DIM, S], bf16, kind="ExternalInput")
    xkT_d = nc.dram_tensor("xkT", [DIM, S], bf16, kind="ExternalInput")
    xvT_d = nc.dram_tensor("xvT", [DIM, S], bf16, kind="ExternalInput")
    wqT_d = nc.dram_tensor("wqT", [DIM, FL], bf16, kind="ExternalInput")
    wkT_d = nc.dram_tensor("wkT", [DIM, FL], bf16, kind="ExternalInput")
    wvT_d = nc.dram_tensor("wvT", [DIM, FL], bf16, kind="ExternalInput")
    woT_d = nc.dram_tensor("woT", [DIM, DIM], bf16, kind="ExternalInput")
    bq_d = nc.dram_tensor("bq", [FL], f32, kind="ExternalInput")
    bk_d = nc.dram_tensor("bk", [FL], f32, kind="ExternalInput")
    bv_d = nc.dram_tensor("bv", [FL], f32, kind="ExternalInput")
    bo_d = nc.dram_tensor("bo", [DIM], f32, kind="ExternalInput")
    gamma_d = nc.dram_tensor("gamma", [DIM], f32, kind="ExternalInput")
    beta_d = nc.dram_tensor("beta", [DIM], f32, kind="ExternalInput")
    y_d = nc.dram_tensor("y", [TH, DIM], f32, kind="ExternalOutput")

    PAIRS = [[0, 1], [2, 3], [4, 5], [6, 7]]

    def bcast_ap(ap, parts):
        return bass.AP(tensor=ap.tensor, offset=ap.offset,
                       ap=[[0, parts]] + list(ap.ap))

    with tile.TileContext(nc) as tc:
        import contextlib
        with contextlib.ExitStack() as ctx:
            persist = ctx.enter_context(tc.tile_pool(name="persist", bufs=1))
            ws = ctx.enter_context(tc.tile_pool(name="ws", bufs=1))
            xs = ctx.enter_context(tc.tile_pool(name="xs", bufs=2))
            et_pool = ctx.enter_context(tc.tile_pool(name="et", bufs=3))
            bc_pool = ctx.enter_context(tc.tile_pool(name="bc", bufs=2))
            ln_pool = ctx.enter_context(tc.tile_pool(name="ln", bufs=2))
            dram = ctx.enter_context(
                tc.tile_pool(name="dram", bufs=1, space="DRAM"))
            scp = ctx.enter_context(
                tc.tile_pool(name="scp", bufs=2, space="PSUM"))
            avp = ctx.enter_context(
                tc.tile_pool(name="avp", bufs=4, space="PSUM"))

            # ---------------- persistent state ----------------
            qhT = persist.tile([HD, NH, S], bf16)
            khT = persist.tile([HD, NH, S], bf16)
            vh_aug = persist.tile([P, JT * VW + (P - HW)], bf16)
            out_normT = persist.tile([P, FL // P, S], bf16)

            wq_sb = ws.tile([P, DC, FL], bf16, tag="wA")
            wk_sb = ws.tile([P, DC, FL], bf16, tag="wB")
            wv_sb = ws.tile([P, DC, FL], bf16, tag="wC")
            # wq on the sync queue (first proj MM needs it + xq tci0);
            # wk/wv on the gpsimd queue so they load in parallel.
            nc.sync.dma_start(wq_sb, wqT_d.rearrange("(a p) f -> p a f", p=P))
            nc.gpsimd.dma_start(wk_sb,
                                wkT_d.rearrange("(a p) f -> p a f", p=P))
            nc.gpsimd.dma_start(wv_sb,
                                wvT_d.rearrange("(a p) f -> p a f", p=P))

            # ones columns (rest of vh_aug holds data or harmless garbage;
            # garbage feeds only psum rows 65:127 which are never read)
            ones_f32 = persist.tile([P, P - HW], f32)
            nc.vector.memset(ones_f32, 1.0)
            vh_view = vh_aug[:, :JT * VW].rearrange("p (j w) -> p j w", w=VW)
            for h in range(NH):
                nc.scalar.copy(vh_view[:, :, h * HW + HD], ones_f32[:, :JT])
            # tail pad after the last head's window must be finite
            nc.scalar.copy(vh_aug[:, JT * VW:], ones_f32)

            bq_sb = persist.tile([P, FL // P], f32)
            bk_sb = persist.tile([P, FL // P], f32)
            nc.sync.dma_start(bq_sb, bq_d.rearrange("(a p) -> p a", p=P))
            nc.sync.dma_start(bk_sb, bk_d.rearrange("(a p) -> p a", p=P))
            bv_bc = persist.tile([P, FL], f32)
            nc.gpsimd.dma_start(bv_bc, bcast_ap(bv_d[:], P))
            bo_bc = persist.tile([P, DIM], f32)
            gamma_bc = persist.tile([P, DIM], f32)
            beta_bc = persist.tile([P, DIM], f32)
            eps_sb = persist.tile([P, 1], f32)
            nc.vector.memset(eps_sb, EPS)
            neg1_sb = persist.tile([P, 1], f32)
            nc.vector.memset(neg1_sb, -1.0)

            scope_stack = []

            def enter_scope(nm):
                while scope_stack:
                    n0, sid = scope_stack.pop()
                    nc.leave_named_scope(n0, sid, False)
                scope_stack.append((nm, nc.enter_named_scope(nm, False)[0]))

            # ---------------- q/k projections ----------------
            enter_scope("proj")
            for name, x_d, w_sb, b_sb, dstT, scl in (
                ("q", xqT_d, wq_sb, bq_sb, qhT, SCALE / 4),
                ("k", xkT_d, wk_sb, bk_sb, khT, 1.0),
            ):
                for tci in range(IC):
                    xt = xs.tile([P, DC, 512], bf16, tag="xqk",
                                 name=f"x{name}_{tci}")
                    for dc in range(DC):
                        nc.sync.dma_start(
                            xt[:, dc, :],
                            x_d[dc * P:(dc + 1) * P,
                                tci * 512:(tci + 1) * 512])
                    for fc in range(FL // P):
                        ps = avp.tile([P, 512], f32, tag="ps512",
                                      name=f"ps_{name}_{tci}_{fc}")
                        for dc in range(DC):
                            nc.tensor.matmul(
                                ps, w_sb[:, dc, fc * P:(fc + 1) * P],
                                xt[:, dc, :],
                                start=(dc == 0), stop=(dc == DC - 1))
                        for hf in range(2):
                            h = 2 * fc + hf
                            nc.scalar.activation(
                                dstT[:, h, tci * 512:(tci + 1) * 512],
                                ps[hf * HD:(hf + 1) * HD], IDENT,
                                bias=b_sb[hf * HD:(hf + 1) * HD, fc:fc + 1],
                                scale=scl)

            # woT reuses the q/k weight slots now that they are free
            woA = ws.tile([P, 4, DIM], bf16, tag="wA", name="woA")
            woB = ws.tile([P, 4, DIM], bf16, tag="wB", name="woB")
            for a in range(4):
                nc.sync.dma_start(woA[:, a, :],
                                  woT_d[a * P:(a + 1) * P, :])
                nc.sync.dma_start(woB[:, a, :],
                                  woT_d[FL + a * P:FL + (a + 1) * P, :])

            # ---------------- v projection ----------------
            vh3 = vh_aug[:, :JT * VW].rearrange(
                "p (j h w) -> p j h w", h=NH, w=HW)
            bv3 = bv_bc.rearrange("p (h w) -> p h w", w=HD)
            for tci in range(IC):
                xvt = xs.tile([P, DC, 512], bf16, tag="xqk",
                              name=f"xv_{tci}")
                for dc in range(DC):
                    nc.sync.dma_start(
                        xvt[:, dc, :],
                        xvT_d[dc * P:(dc + 1) * P,
                              tci * 512:(tci + 1) * 512])
                for tj in range(4):
                    jt = tci * 4 + tj
                    ps = avp.tile([P, FL], f32, tag="ps512",
                                  name=f"ps_v_{jt}")
                    for dc in range(DC):
                        nc.tensor.matmul(
                            ps, xvt[:, dc, tj * P:(tj + 1) * P],
                            wv_sb[:, dc, :],
                            start=(dc == 0), stop=(dc == DC - 1))
                    nc.vector.scalar_tensor_tensor(
                        vh3[:, jt, :, :HD],
                        ps.rearrange("p (h w) -> p h w", w=HD), 0.0,
                        bv3, op0=ALU.add, op1=ALU.add)

            # ---------------- attention ----------------
            # half=1 (peer token half, cols TH:S) first so the pair exchange
            # overlaps the half=0 attention.
            cc_in = dram.tile([FL, TH], bf16)
            cc_sum = dram.tile([FL, TH], bf16)

            def attn_head(half, h):
                colb = half * TH
                pav = [avp.tile([P, 512], f32, tag="ps512",
                                name=f"pav_{half}_{h}_{i}") for i in range(2)]
                et_t = {}

                def S_step(jt):
                    psc = scp.tile([P, 1024], f32, tag="sc",
                                   name=f"sc_{half}_{h}_{jt}")
                    for i2 in range(2):
                        nc.tensor.matmul(
                            psc[:, i2 * 512:(i2 + 1) * 512],
                            khT[:, h, jt * P:(jt + 1) * P],
                            qhT[:, h, colb + i2 * 512:colb + (i2 + 1) * 512],
                            start=True, stop=True)
                    et = et_pool.tile([P, 1024], bf16, tag="et",
                                      name=f"et_{half}_{h}_{jt}")
                    if jt in (6, 10, 14):
                        nc.vector._custom_dve(exp_op, out=et, in0=psc,
                                              s0=EXP_C2, s1=EXP_C3)
                    else:
                        nc.scalar.activation(et, psc, EXPF, scale=4.0)
                    et_t[jt] = et

                def AV_step(jt):
                    et = et_t.pop(jt)
                    for i2 in range(2):
                        nc.tensor.matmul(
                            pav[i2],
                            vh_aug[:, jt * VW + h * HW:jt * VW + h * HW + P],
                            et[:, i2 * 512:(i2 + 1) * 512],
                            start=(jt == 0), stop=(jt == JT - 1))

                S_step(0)
                S_step(1)
                for jt in range(JT):
                    if jt + 2 < JT:
                        S_step(jt + 2)
                    AV_step(jt)

                # evict + normalize: row 64 holds the softmax denominator
                hb = (h % 2) * HD
                for i2 in range(2):
                    rr = bc_pool.tile([1, 2, 512], f32, tag="rr",
                                      name=f"rr_{half}_{h}_{i2}")
                    # custom-DVE recip can't read PSUM: stage row to SBUF
                    nc.vector.tensor_copy(rr[:, 1, :], pav[i2][HD:HD + 1])
                    nc.vector.reciprocal_approx_fast(rr[:, 0, :], rr[:, 1, :])
                    rb = bc_pool.tile([HD, 512], f32, tag="rb",
                                      name=f"rb_{half}_{h}_{i2}")
                    nc.gpsimd.partition_broadcast(rb, rr[:, 0, :])
                    nc.vector.scalar_tensor_tensor(
                        out_normT[hb:hb + HD, h // 2,
                                  colb + i2 * 512:colb + (i2 + 1) * 512],
                        pav[i2][:HD], 0.0, rb, op0=ALU.add, op1=ALU.mult)

            enter_scope("attnP")
            for h in range(NH):
                attn_head(1, h)

            # ---------------- pair exchange (fires under attnM) ----------
            enter_scope("exch")
            for pi in range(FL // P):
                nc.sync.dma_start(cc_in[pi * P:(pi + 1) * P, :],
                                  out_normT[:, pi, TH:S])
            nc.gpsimd.collective_compute(
                "AllReduce", ALU.add, replica_groups=PAIRS,
                ins=[cc_in.opt()], outs=[cc_sum.opt()])

            nc.gpsimd.dma_start(bo_bc, bcast_ap(bo_d[:], P))
            nc.gpsimd.dma_start(gamma_bc, bcast_ap(gamma_d[:], P))
            nc.gpsimd.dma_start(beta_bc, bcast_ap(beta_d[:], P))

            enter_scope("attnM2")
            peer_T = None
            for h in range(NH):
                attn_head(0, h)
                if h == 4:
                    # peer = (mine + peer) - mine; emitted this late so the
                    # collective is long done and the DVE never head-of-line
                    # blocks the eviction chain (a stall here re-throttles
                    # the PE clock for the rest of the kernel).
                    peer_T = ws.tile([P, FL // P, TH], bf16, tag="wC")
                    ta_all = xs.tile([P, FL // P, TH], bf16, tag="xqk",
                                     name="cc_stage")
                    nc.sync.dma_start(
                        ta_all,
                        cc_sum[:, :].rearrange("(a p) t -> p a t", p=P))
                    for pi in range(FL // P):
                        nc.vector.scalar_tensor_tensor(
                            peer_T[:, pi, :], ta_all[:, pi, :], 0.0,
                            out_normT[:, pi, TH:S],
                            op0=ALU.add, op1=ALU.subtract)

            # ---------------- output projection + layernorm ----------------
            enter_scope("final")
            for it in range(TH // P):
                psf = [avp.tile([P, 512], f32, tag="ps512",
                                name=f"psf_{it}_{f}") for f in range(2)]
                for fcc in range(2):
                    for cc in range(DC):
                        if cc < 4:
                            stat = out_normT[:, cc, it * P:(it + 1) * P]
                            mov = woA[:, cc, fcc * 512:(fcc + 1) * 512]
                        else:
                            stat = peer_T[:, cc - 4, it * P:(it + 1) * P]
                            mov = woB[:, cc - 4, fcc * 512:(fcc + 1) * 512]
                        nc.tensor.matmul(
                            psf[fcc], stat, mov,
                            start=(cc == 0), stop=(cc == DC - 1))
                xln = ln_pool.tile([P, DIM], f32, tag="xln", name=f"xln_{it}")
                if trivial_ln:
                    # gamma==1, beta==0 (checked host-side): mean rides the
                    # bo-add accum; sumsq via ACT Square; one ACT does the
                    # whole normalize y = xln*rstd - mu*rstd.
                    xsq = ln_pool.tile([P, DIM], f32, tag="xsq",
                                       name=f"xsq_{it}")
                    sm = ln_pool.tile([P, 8], f32, tag="sm", name=f"sm_{it}")
                    for fcc in range(2):
                        nc.vector.scalar_tensor_tensor(
                            xln[:, fcc * 512:(fcc + 1) * 512], psf[fcc], 0.0,
                            bo_bc[:, fcc * 512:(fcc + 1) * 512],
                            op0=ALU.add, op1=ALU.add,
                            accum_out=sm[:, fcc:fcc + 1])
                    nc.scalar.activation(xsq, xln, SQUARE,
                                         accum_out=sm[:, 2:3])
                    nc.vector.tensor_tensor(sm[:, 3:4], sm[:, 0:1],
                                            sm[:, 1:2], op=ALU.add)
                    nc.vector.tensor_single_scalar(
                        sm[:, 3:4], sm[:, 3:4], 1.0 / DIM, op=ALU.mult)
                    nc.vector.tensor_tensor(sm[:, 4:5], sm[:, 3:4],
                                            sm[:, 3:4], op=ALU.mult)
                    nc.vector.scalar_tensor_tensor(
                        sm[:, 5:6], sm[:, 2:3], 1.0 / DIM, sm[:, 4:5],
                        op0=ALU.mult, op1=ALU.subtract)
                    nc.scalar.activation(sm[:, 5:6], sm[:, 5:6], SQRTF,
                                         bias=eps_sb)
                    nc.vector.reciprocal(sm[:, 5:6], sm[:, 5:6])
                    nc.vector.scalar_tensor_tensor(
                        sm[:, 6:7], sm[:, 3:4], sm[:, 5:6], neg1_sb,
                        op0=ALU.mult, op1=ALU.mult)
                    nc.scalar.activation(xsq, xln, IDENT,
                                         bias=sm[:, 6:7], scale=sm[:, 5:6])
                    nc.sync.dma_start(y_d[it * P:(it + 1) * P, :], xsq)
                    continue
                for fcc in range(2):
                    nc.vector.scalar_tensor_tensor(
                        xln[:, fcc * 512:(fcc + 1) * 512], psf[fcc], 0.0,
                        bo_bc[:, fcc * 512:(fcc + 1) * 512],
                        op0=ALU.add, op1=ALU.add)
                stats = ln_pool.tile([P, 2, 6], f32, tag="st", name=f"st_{it}")
                for hf in range(2):
                    nc.vector.bn_stats(stats[:, hf, :],
                                       xln[:, hf * 512:(hf + 1) * 512])
                mv = ln_pool.tile([P, 2], f32, tag="mv", name=f"mv_{it}")
                nc.vector.bn_aggr(mv, stats)
                rstd = ln_pool.tile([P, 1], f32, tag="rstd", name=f"rstd_{it}")
                nc.scalar.activation(rstd, mv[:, 1:2], SQRTF, bias=eps_sb)
                nc.vector.reciprocal(rstd, rstd)
                nc.vector.scalar_tensor_tensor(
                    xln, xln, mv[:, 0:1], gamma_bc,
                    op0=ALU.subtract, op1=ALU.mult)
                nc.vector.scalar_tensor_tensor(
                    xln, xln, rstd, beta_bc, op0=ALU.mult, op1=ALU.add)
                nc.sync.dma_start(y_d[it * P:(it + 1) * P, :], xln)
            while scope_stack:
                n0, sid = scope_stack.pop()
                nc.leave_named_scope(n0, sid, False)

    nc.compile()
    return nc


def kernel(q, k, v, Wq, bq, Wk, bk, Wv, bv, Wo, bo, gamma, beta):
    from concourse.bass_utils import run_bass_kernel_spmd

    if "nc" not in _cache:
        _cache["nc"] = _build(trivial_ln=False)
    nc = _cache["nc"]

    q = np.asarray(q, np.float32)
    k = np.asarray(k, np.float32)
    v = np.asarray(v, np.float32)
    Wq = np.asarray(Wq, np.float32); Wk = np.asarray(Wk, np.float32)
    Wv = np.asarray(Wv, np.float32); Wo = np.asarray(Wo, np.float32)
    bf = ml_dtypes.bfloat16
    WoT = Wo.T  # [c, f]
    in_maps = []
    for c in range(NCORES):
        b, g = c // 2, c % 2
        fsl = slice(g * FL, (g + 1) * FL)
        # my token half first (so "my half" is always columns 0:TH)
        xq = q[b].T if g == 0 else np.concatenate(
            [q[b].T[:, TH:], q[b].T[:, :TH]], axis=1)
        # Wo rows rotated: [my features, peer features]
        woT = np.concatenate([WoT[g * FL:(g + 1) * FL],
                              WoT[(1 - g) * FL:(2 - g) * FL]], axis=0)
        in_maps.append({
            "xqT": np.ascontiguousarray(xq).astype(bf),
            "xkT": np.ascontiguousarray(k[b].T).astype(bf),
            "xvT": np.ascontiguousarray(v[b].T).astype(bf),
            "wqT": np.ascontiguousarray(Wq[fsl, :].T).astype(bf),
            "wkT": np.ascontiguousarray(Wk[fsl, :].T).astype(bf),
            "wvT": np.ascontiguousarray(Wv[fsl, :].T).astype(bf),
            "woT": np.ascontiguousarray(woT).astype(bf),
            "bq": np.asarray(bq, np.float32)[fsl] * (SCALE / 4),
            "bk": np.asarray(bk, np.float32)[fsl],
            "bv": np.asarray(bv, np.float32)[fsl],
            "bo": np.asarray(bo, np.float32),
            "gamma": np.asarray(gamma, np.float32),
            "beta": np.asarray(beta, np.float32),
        })
    res = run_bass_kernel_spmd(nc, in_maps, list(range(NCORES)),
                               trace=_cache.get("trace", False))
    _cache["last_res"] = res
    y = np.empty((B, S, DIM), np.float32)
    for c in range(NCORES):
        b, g = c // 2, c % 2
        y[b, g * TH:(g + 1) * TH, :] = res.results[c]["y"]
    return y

